# revision 15
# baseline (speedup 1.0000x reference)
"""Trainium2 Bass kernel for nn_AblatedModel_40802189312754 (2-layer GNN + scoring).

Sharding: entities row-sharded 8 ways (6250/core, padded to 6400); batch
replicated; final [B, N] logits column-sharded by entity shard.

Per core: SpMM = (edge-feature stream) x (host-built per-chunk indicator
matrices) accumulated as PE matmuls into PSUM windows of 512 segments,
everything kept transposed (dim on partitions). Layer-1 edge features are
host-pre-gathered (input indexing) and streamed; layer-2 edge features are
dma_gather'ed from the AllGathered bf16 h-table. All 8 cores share one
instruction stream, so the chunk grid is uniform: bins of 32 segments,
chunk count per (window, bin, col-half) = max over cores. Scoring GEMM fp32.
"""
import sys
sys.path.insert(0, '/opt/trn_rl_repo')

import numpy as np
import ml_dtypes

import concourse.bacc as bacc
import concourse.tile as tile
import concourse.mybir as mybir
from concourse.bass_utils import run_bass_kernel_spmd

BF16 = ml_dtypes.bfloat16

N_ENT = 50000
D = 128
B = 1024
NC = 8
SH = 6250            # real entities per shard
NSH = 6400           # padded shard size (50 x 128)
NV = NC * NSH        # virtual table rows (51200)
BN_EPS = 1e-5
SEGW = 32            # indicator width == bin size in segments
WINDOWS = [(w, min(512, NSH - w)) for w in range(0, NSH, 512)]  # 13 windows
NBIN = NSH // SEGW   # 200 bins of 32 segs per core
HALF = 32768


def _vid(ent):
    owner = ent // SH
    return owner * NSH + (ent - owner * SH)


def _wrap_idx(ids):
    """[n] -> [128, n//16] int16 gather-index layout (wrapped, replicated 8x)."""
    n = len(ids)
    w = ids.reshape(n // 16, 16).T
    return np.ascontiguousarray(np.tile(w, (8, 1)).astype(np.int16))


def _make_plan(rows, cols, vals):
    """Uniform cross-core plan.

    Returns (struct, cores):
      struct['win'][wi] = {'chunks': [(seg0, slot), ...], 'groups': [half,...]}
      cores[k] = {'idx': [128, NG*64] i16, 'ind': [128, NCH*SEGW] f32,
                  'g1src': [NCH, 128] original col id or -1}
    """
    vcol_all = _vid(cols)
    per_core = []
    for k in range(NC):
        m = (rows >= k * SH) & (rows < (k + 1) * SH)
        r = rows[m] - k * SH
        vc = vcol_all[m]
        c = cols[m]
        v = vals[m].astype(np.float32)
        half = (vc >= HALF).astype(np.int64)
        key = (r // SEGW) * 2 + half
        o = np.lexsort((vc, r, key))
        per_core.append((key[o], r[o], vc[o], c[o], v[o]))

    # chunk count per (bin, half): max over cores, then per-(window,half)
    # totals rounded up to a multiple of 8 (gather group size)
    nch = np.zeros((NBIN, 2), np.int64)
    bounds = []
    for k in range(NC):
        key = per_core[k][0]
        lo = np.searchsorted(key, np.arange(NBIN * 2))
        hi = np.searchsorted(key, np.arange(NBIN * 2) + 1)
        bounds.append((lo, hi))
        cnt = (hi - lo).reshape(NBIN, 2)
        nch = np.maximum(nch, -(-cnt // 128))
    struct = {'win': []}
    core_chunks = [[] for _ in range(NC)]   # per core: (a, b) edge ranges
    for wi, (w0, wsz) in enumerate(WINDOWS):
        b0, b1 = w0 // SEGW, (w0 + wsz) // SEGW
        chunks, groups = [], []
        for half in (0, 1):
            cl = [(b, j) for b in range(b0, b1) for j in range(nch[b, half])]
            ng = -(-len(cl) // 8)
            base = len(groups) * 8
            groups += [half] * ng
            for ci, (b, j) in enumerate(cl):
                chunks.append(((b - b0) * SEGW, base + ci))
                for k in range(NC):
                    lo, hi = bounds[k]
                    a = lo[b * 2 + half] + j * 128
                    e = min(a + 128, hi[b * 2 + half])
                    core_chunks[k].append((a, max(a, e)))
            base += ng * 8 - len(cl)  # skip pad slots at end of half
        struct['win'].append({'chunks': chunks, 'groups': groups})

    # per-core blobs in global chunk order / global group order
    ncht = sum(len(w['chunks']) for w in struct['win'])
    ngt = sum(len(w['groups']) for w in struct['win'])
    cores = []
    for k in range(NC):
        key, r, vc, c, v = per_core[k]
        ind = np.zeros((ncht, 128, SEGW), np.float32)
        g1src = np.full((ncht, 128), -1, np.int64)
        idx = np.zeros((ngt, 1024), np.int64)
        ci = 0
        gi = 0
        for wi, (w0, wsz) in enumerate(WINDOWS):
            w = struct['win'][wi]
            for (seg0, slot) in w['chunks']:
                a, e = core_chunks[k][ci]
                n = e - a
                if n:
                    ind[ci, np.arange(n), r[a:e] - w0 - seg0] = v[a:e]
                    g1src[ci, :n] = c[a:e]
                    g = gi + slot // 8
                    off = (slot % 8) * 128
                    h = w['groups'][slot // 8]
                    idx[g, off:off + n] = vc[a:e] - (HALF if h else 0)
                ci += 1
            gi += len(w['groups'])
        cores.append({
            'idx': np.concatenate([_wrap_idx(idx[g]) for g in range(ngt)], 1),
            'ind': np.ascontiguousarray(
                ind.transpose(1, 0, 2).reshape(128, ncht * SEGW)),
            'g1src': g1src,
        })
    return struct, cores


def _build_nc(struct):
    ngt = sum(len(w['groups']) for w in struct['win'])
    ncht = sum(len(w['chunks']) for w in struct['win'])
    maxg = max(len(w['groups']) for w in struct['win'])
    maxch = max(len(w['chunks']) for w in struct['win'])

    nc = bacc.Bacc("TRN2", target_bir_lowering=False, debug=False,
                   enable_asserts=True, num_devices=NC, num_swdge_queues=4)
    f32, bf, i16 = mybir.dt.float32, mybir.dt.bfloat16, mybir.dt.int16
    AF = mybir.ActivationFunctionType

    g1_d = nc.dram_tensor("g1", [128, ngt * 8 * 128], bf, kind="ExternalInput")
    ind_d = nc.dram_tensor("ind", [128, ncht * SEGW], bf, kind="ExternalInput")
    idx_d = nc.dram_tensor("idx", [128, ngt * 64], i16, kind="ExternalInput")
    w1_d = nc.dram_tensor("w1", [D, D], f32, kind="ExternalInput")
    w2_d = nc.dram_tensor("w2", [D, D], f32, kind="ExternalInput")
    w_d = nc.dram_tensor("w", [D, D], f32, kind="ExternalInput")
    bn_d = nc.dram_tensor("bn", [D, 8], f32, kind="ExternalInput")
    et_d = nc.dram_tensor("et", [128, NSH], f32, kind="ExternalInput")
    ebh_d = nc.dram_tensor("ebh", [128, B], f32, kind="ExternalInput")
    rgt_d = nc.dram_tensor("rgt", [128, B], f32, kind="ExternalInput")
    bidx_d = nc.dram_tensor("bidx", [128, 64], i16, kind="ExternalInput")
    ident_d = nc.dram_tensor("ident", [D, D], f32, kind="ExternalInput")
    out_d = nc.dram_tensor("out", [8, 128, NSH], f32, kind="ExternalOutput")

    with tile.TileContext(nc) as tc:
        with tc.tile_pool(name="const", bufs=1) as cp, \
             tc.tile_pool(name="gp", bufs=2) as gp, \
             tc.tile_pool(name="indp", bufs=2) as indp, \
             tc.tile_pool(name="idxp", bufs=1) as idxp, \
             tc.tile_pool(name="sp", bufs=1) as spool, \
             tc.tile_pool(name="hp", bufs=1) as hp, \
             tc.tile_pool(name="hep", bufs=4) as hep, \
             tc.tile_pool(name="bp", bufs=1) as bp, \
             tc.tile_pool(name="op", bufs=3) as op, \
             tc.tile_pool(name="pch", bufs=2, space="PSUM") as pch, \
             tc.tile_pool(name="px", bufs=2, space="PSUM") as px, \
             tc.tile_pool(name="dram", bufs=2, space="DRAM") as dp:

            w1_t = cp.tile([D, D], f32); nc.sync.dma_start(w1_t[:], w1_d[:])
            w2_t = cp.tile([D, D], f32); nc.sync.dma_start(w2_t[:], w2_d[:])
            w_t = cp.tile([D, D], f32); nc.sync.dma_start(w_t[:], w_d[:])
            bn_t = cp.tile([D, 8], f32); nc.sync.dma_start(bn_t[:], bn_d[:])
            id_t = cp.tile([D, D], f32); nc.sync.dma_start(id_t[:], ident_d[:])
            ebh_t = cp.tile([128, B], f32); nc.sync.dma_start(ebh_t[:], ebh_d[:])
            rgt_t = cp.tile([128, B], f32); nc.sync.dma_start(rgt_t[:], rgt_d[:])
            bidx_t = cp.tile([128, 64], i16); nc.sync.dma_start(bidx_t[:], bidx_d[:])
            zl_t = cp.tile([1, 128], bf); nc.vector.memset(zl_t[:], 0.0)
            zr_t = cp.tile([1, 512], bf); nc.vector.memset(zr_t[:], 0.0)

            idx_all = idxp.tile([128, ngt * 64], i16)
            nc.sync.dma_start(idx_all[:], idx_d[:])
            s_t = spool.tile([128, NSH], f32, tag="s")
            h1t_t = hp.tile([128, NSH], bf, tag="h1t")
            h2t_t = hp.tile([128, NSH], f32, tag="h2t")
            h2tb_t = hp.tile([128, NSH], bf, tag="h2tb")

            hsh = [dp.tile([NSH, D], bf, tag="hsh", name=f"hsh{i}")
                   for i in range(2)]
            hfull = dp.tile([NV, D], bf, tag="hfull", addr_space="Shared",
                            name="hfull0")

            def spmm_layer(layer):
                gbase, cbase, qn = 0, 0, 0
                for wi, (w0, wsz) in enumerate(WINDOWS):
                    meta = struct['win'][wi]
                    ngr, nch = len(meta['groups']), len(meta['chunks'])
                    g_t = gp.tile([128, maxg * 1024], bf, tag="g")
                    ind_t = indp.tile([128, maxch * SEGW], bf, tag="ind")
                    nc.sync.dma_start(
                        ind_t[:, :nch * SEGW],
                        ind_d[:, cbase * SEGW:(cbase + nch) * SEGW])
                    if layer == 0:
                        nc.sync.dma_start(
                            g_t[:, :ngr * 1024],
                            g1_d[:, gbase * 1024:(gbase + ngr) * 1024])
                    else:
                        for gi, half in enumerate(meta['groups']):
                            src = hfull[HALF:NV] if half else hfull[0:HALF]
                            nc.gpsimd.dma_gather(
                                g_t[:, gi * 1024:(gi + 1) * 1024]
                                    .rearrange("p (c e) -> p c e", e=D),
                                src, idx_all[:, (gbase + gi) * 64:
                                             (gbase + gi + 1) * 64],
                                1024, 1024, D, queue_num=qn % 4,
                                single_packet=False)
                            qn += 1
                    ps = pch.tile([128, 512], f32, tag="ps")
                    nc.tensor.matmul(ps[:], zl_t[:], zr_t[:],
                                     start=True, stop=False, skip_group_check=True)
                    for ci, (seg0, slot) in enumerate(meta['chunks']):
                        nc.tensor.matmul(
                            ps[:, seg0:seg0 + SEGW],
                            g_t[:, slot * 128:(slot + 1) * 128],
                            ind_t[:, ci * SEGW:(ci + 1) * SEGW],
                            start=False, stop=(ci == nch - 1),
                            skip_group_check=True)
                    nc.vector.tensor_copy(s_t[:, w0:w0 + wsz], ps[:, :wsz])
                    gbase += ngr
                    cbase += nch

            def xform(layer):
                wmat = w1_t if layer == 0 else w2_t
                bcol = bn_t[:, 0:1] if layer == 0 else bn_t[:, 1:2]
                for (w0, wsz) in WINDOWS:
                    xp = px.tile([128, 512], f32, tag="xp")
                    nc.tensor.matmul(xp[:, :wsz], wmat[:], s_t[:, w0:w0 + wsz],
                                     start=True, stop=True)
                    if layer == 0:
                        nc.scalar.activation(h1t_t[:, w0:w0 + wsz], xp[:, :wsz],
                                             AF.Relu, bias=bcol, scale=1.0)
                    else:
                        nc.scalar.activation(h2t_t[:, w0:w0 + wsz], xp[:, :wsz],
                                             AF.Relu, bias=bcol, scale=1.0)
                        nc.vector.tensor_copy(h2tb_t[:, w0:w0 + wsz],
                                              h2t_t[:, w0:w0 + wsz])

            def store_table(src_bf, li, gather_full):
                for t in range(NSH // 128):
                    hent = hep.tile([128, 128], bf, tag="hent")
                    nc.sync.dma_start_transpose(
                        hent[:], src_bf[:, t * 128:(t + 1) * 128])
                    nc.sync.dma_start(hsh[li][t * 128:(t + 1) * 128, :], hent[:])
                if gather_full:
                    nc.gpsimd.collective_compute(
                        "AllGather", mybir.AluOpType.bypass,
                        replica_groups=[list(range(NC))],
                        ins=[hsh[li][:].opt()], outs=[hfull[:].opt()])

            spmm_layer(0)
            xform(0)
            store_table(h1t_t, 0, True)
            spmm_layer(1)
            # zero h2 row NSH-1 so out-of-shard batch slots contribute 0
            xform(1)
            nc.vector.memset(h2tb_t[:, NSH - 1:NSH], 0.0)
            store_table(h2tb_t, 1, False)

            # batch tail: gather own-shard batch rows from the local h2 table,
            # AllReduce the partials (out-of-shard slots hit the zero row)
            tlo = bp.tile([128, B], bf)
            nc.gpsimd.dma_gather(
                tlo[:].rearrange("p (c e) -> p c e", e=D), hsh[1][:],
                bidx_t[:, 0:64], 1024, 1024, D, queue_num=0,
                single_packet=False)
            xpart = bp.tile([128, B], f32)
            nc.vector.tensor_copy(xpart[:], tlo[:])
            xin_dram = dp.tile([128, B], f32, tag="xin")
            xout_dram = dp.tile([128, B], f32, tag="xout", addr_space="Shared")
            nc.sync.dma_start(xin_dram[:], xpart[:])
            nc.gpsimd.collective_compute(
                "AllReduce", mybir.AluOpType.add,
                replica_groups=[list(range(NC))],
                ins=[xin_dram[:].opt()], outs=[xout_dram[:].opt()])
            xraw = bp.tile([128, B], f32)
            nc.sync.dma_start(xraw[:], xout_dram[:])
            nc.vector.tensor_tensor(xraw[:], xraw[:], ebh_t[:],
                                    mybir.AluOpType.add)
            xtb = bp.tile([128, B], f32)
            for j in range(8):
                tp = px.tile([128, 128], f32, tag="tp")
                nc.tensor.transpose(tp[:], xraw[:, j * 128:(j + 1) * 128], id_t[:])
                nc.vector.tensor_scalar(
                    xtb[:, j * 128:(j + 1) * 128], tp[:],
                    bn_t[:, 2:3], bn_t[:, 3:4],
                    mybir.AluOpType.mult, mybir.AluOpType.add)
            vmt = bp.tile([128, B], f32)
            for hb in range(2):
                sl = slice(hb * 512, hb * 512 + 512)
                wmp = px.tile([128, 512], f32, tag="xp")
                nc.tensor.matmul(wmp[:], w_t[:], rgt_t[:, sl],
                                 start=True, stop=True)
                nc.vector.tensor_tensor(vmt[:, sl], xtb[:, sl], wmp[:],
                                        mybir.AluOpType.mult)
            nc.vector.tensor_scalar(vmt[:], vmt[:], bn_t[:, 4:5], bn_t[:, 5:6],
                                    mybir.AluOpType.mult, mybir.AluOpType.add)

            # scoring: fet = E^T + h2^T (reuses the s slot; et loaded in place)
            fet = spool.tile([128, NSH], f32, tag="s")
            nc.sync.dma_start(fet[:], et_d[:])
            nc.vector.tensor_tensor(fet[:], fet[:], h2t_t[:], mybir.AluOpType.add)
            for bt in range(8):
                for (w0, wsz) in WINDOWS:
                    sc = pch.tile([128, 512], f32, tag="ps")
                    nc.tensor.matmul(sc[:, :wsz], vmt[:, bt * 128:(bt + 1) * 128],
                                     fet[:, w0:w0 + wsz], start=True, stop=True)
                    ob = op.tile([128, 512], f32, tag="ob")
                    nc.scalar.activation(ob[:, :wsz], sc[:, :wsz], AF.Sigmoid)
                    nc.sync.dma_start(out_d[bt, :, w0:w0 + wsz], ob[:, :wsz])
    nc.compile()
    return nc


def _host_prep(inputs):
    rows = np.asarray(inputs["adj_rows"]).astype(np.int64)
    cols = np.asarray(inputs["adj_cols"]).astype(np.int64)
    vals = np.asarray(inputs["adj_vals"], np.float32)
    E = np.asarray(inputs["E_emb"], np.float32)[np.asarray(inputs["init_ind"])]
    E_bf = E.astype(BF16)
    bh = np.asarray(inputs["batch_head"]).astype(np.int64)
    rel = np.asarray(inputs["batch_rel"]).astype(np.int64)
    R = np.asarray(inputs["R_emb"], np.float32)

    g0 = np.asarray(inputs["bn0_gamma"], np.float32) / np.sqrt(1.0 + BN_EPS)
    b0 = np.asarray(inputs["bn0_beta"], np.float32)
    g1 = np.asarray(inputs["bn1_gamma"], np.float32) / np.sqrt(1.0 + BN_EPS)
    b1v = np.asarray(inputs["bn1_beta"], np.float32)
    bn = np.ascontiguousarray(np.stack(
        [np.asarray(inputs["b1"], np.float32),
         np.asarray(inputs["b2"], np.float32),
         g0, b0, g1, b1v,
         np.zeros(D, np.float32), np.zeros(D, np.float32)], axis=1))

    bh_owner = bh // SH
    bh_local = bh - bh_owner * SH

    def slot_layout(a):          # [1024, D] -> [128, 8*D], slot i=(p,j)->j*128+p
        return np.ascontiguousarray(
            a.reshape(8, 128, D).transpose(1, 0, 2).reshape(128, 8 * D))

    ebh_l = slot_layout(E[bh])
    rgt = np.ascontiguousarray(R[rel].T.astype(np.float32))

    struct, cores = _make_plan(rows, cols, vals)
    ngt = sum(len(w['groups']) for w in struct['win'])

    in_maps = []
    for k in range(NC):
        pl = cores[k]
        # g1 blob: slot-major [128, ngt*8*128]
        g1_cols = np.zeros((128, ngt * 8 * 128), BF16)
        ci = 0
        gbase = 0
        for w in struct['win']:
            for (seg0, slot) in w['chunks']:
                src = pl['g1src'][ci]
                m = src >= 0
                blk = np.zeros((128, D), BF16)
                blk[m] = E_bf[src[m]]
                c0 = (gbase * 8 + slot) * 128
                g1_cols[:, c0:c0 + 128] = blk
                ci += 1
            gbase += len(w['groups'])
        et = np.zeros((D, NSH), np.float32)
        et[:, :SH] = E[k * SH:(k + 1) * SH].T
        in_maps.append({
            "g1": g1_cols,
            "ind": pl['ind'].astype(BF16),
            "idx": pl['idx'],
            "w1": np.asarray(inputs["W1"], np.float32),
            "w2": np.asarray(inputs["W2"], np.float32),
            "w": np.asarray(inputs["W"], np.float32),
            "bn": bn, "et": et, "ebh": ebh_l, "rgt": rgt,
            "bidx": _wrap_idx(np.where(bh_owner == k, bh_local, NSH - 1)),
            "ident": np.eye(D, dtype=np.float32),
        })
    return struct, in_maps


def _run(inputs, trace=False):
    struct, in_maps = _host_prep(inputs)
    nc = _build_nc(struct)
    res = run_bass_kernel_spmd(nc, in_maps, core_ids=list(range(NC)),
                               trace=trace)
    outs = [res.results[k]["out"].reshape(B, NSH)[:, :SH] for k in range(NC)]
    return np.concatenate(outs, axis=1).astype(np.float32), res


def kernel(**inputs):
    out, _ = _run(inputs, trace=False)
    return out
